# revision 28
# baseline (speedup 1.0000x reference)
"""Trainium2 Bass kernel for nn_Attention_81655918231876.

RoPE attention with positional bias, 8 heads / dim_head 64, b=2, n=2048, dim=512.
Sharding: head-parallel across 8 cores. Core h computes head h for BOTH batches
and emits a partial output y_h = softmax(q_h k_h^T + bias_h) v_h @ w_out[h-slice].
The host sums the 8 partials.

v2 design (all-bf16 matmul path; ~50x accuracy headroom under the 2e-2 gate):
  - Projections: stationary weight blocks [q|qrot], [k|krot], [v|pad] in bf16,
    moving x^T chunks; RoPE combine = one DVE mul with a stacked cos/sin table
    plus one DVE add that writes bf16 q/k packs ([b0;b1] on partitions).
  - S = q k^T as plain bf16 K=64 matmuls (tile_position rows 0/64 pick the
    batch half of the packed q/k tiles).
  - exp(S) on ScalarE -> bf16; bias multiply exp(S)*exp(bias) on DVE at 2x
    bf16 rate against a resident bf16 exp(bias^T) table loaded once.
  - P V accumulated with an extra ones-column in V so row 64 of O^T is the
    softmax row sum; 1/sum broadcast across partitions (gpsimd) and folded
    into O^T before the output projection; y DMA'd straight from PSUM.
  - V natural layout built with DMA xbar transposes (no PE transposes).
  - S -> exp -> mult -> PV software-pipelined with a 2-step lag so the PE
    stream never waits on ScalarE/DVE (keeps the PE p-state at full clock).
"""

import numpy as np
import ml_dtypes
import sys

sys.path.insert(0, "/opt/trn_rl_repo")

HEADS = 8
DIM_HEAD = 64
ROPE_THETA = 10000.0
B, N, DIM = 2, 2048, 512
# per-j-block column stride in vsb: 64 V cols + 1 ones col + pad. Must keep
# every block's byte offset 32B-aligned: the DMA xbar transpose writes in
# 16-element (bf16) groups and silently corrupts unaligned destinations.
VSTRIDE = 80

_compiled = None
_DEBUG = False


def _build():
    import concourse.bass as bass
    import concourse.tile as tile
    from concourse import bacc, mybir

    f32 = mybir.dt.float32
    bf16 = mybir.dt.bfloat16
    Exp = mybir.ActivationFunctionType.Exp
    Copy = mybir.ActivationFunctionType.Copy

    nc = bacc.Bacc(None, target_bir_lowering=False, debug=False)
    xt = nc.dram_tensor("xt", [DIM, 2 * N], bf16, kind="ExternalInput")
    wall = nc.dram_tensor("wall", [DIM, 384], bf16, kind="ExternalInput")
    cs2 = nc.dram_tensor("cs2", [128, N], f32, kind="ExternalInput")
    ebt = nc.dram_tensor("ebt", [N, N], bf16, kind="ExternalInput")
    wo = nc.dram_tensor("wo", [64, DIM], bf16, kind="ExternalInput")
    out = nc.dram_tensor("out", [B, N, DIM], bf16, kind="ExternalOutput")
    rsum = nc.dram_tensor("rsum", [4 * B, 512], f32, kind="ExternalOutput")
    if _DEBUG:
        dbg_qkv = nc.dram_tensor("dbg_qkv", [3, 128, N], bf16, kind="ExternalOutput")
        dbg_vsb = nc.dram_tensor("dbg_vsb", [B, 128, 16 * VSTRIDE], bf16, kind="ExternalOutput")
        dbg_pt = nc.dram_tensor("dbg_pt", [128, 1024], bf16, kind="ExternalOutput")
        dbg_rb = nc.dram_tensor("dbg_rb", [64, 512], f32, kind="ExternalOutput")

    with tile.TileContext(nc) as tc:
        with (
            tc.tile_pool(name="singles", bufs=1) as singles,
            tc.tile_pool(name="t12p", bufs=3) as t12p,
            tc.tile_pool(name="ptsp", bufs=3) as ptsp,
            tc.tile_pool(name="ptp", bufs=6) as ptp,
            tc.tile_pool(name="rrp", bufs=2) as rrp,
            tc.tile_pool(name="otp", bufs=2) as otp,
            tc.tile_pool(name="ysp", bufs=3) as ysp,
        ):
            # ---- constants / inputs ----
            wl = [singles.tile([128, 384], bf16, tag=f"wl{k}", name=f"wl{k}") for k in range(4)]
            for k in range(4):
                nc.sync.dma_start(out=wl[k], in_=wall[128 * k:128 * (k + 1), :])
            xb = [singles.tile([128, 2 * N], bf16, tag=f"xb{k}", name=f"xb{k}") for k in range(4)]
            for half in range(2):
                for k in range(4):
                    nc.sync.dma_start(
                        out=xb[k][:, N * half:N * (half + 1)],
                        in_=xt[128 * k:128 * (k + 1), N * half:N * (half + 1)],
                    )
            cs_sb = singles.tile([128, N], f32, tag="cs", name="cs_sb")
            nc.sync.dma_start(out=cs_sb, in_=cs2[:, :])
            wo_sb = singles.tile([64, DIM], bf16, tag="wo", name="wo_sb")
            nc.sync.dma_start(out=wo_sb, in_=wo[:, :])
            eb_sb = singles.tile([128, 16 * N], bf16, tag="eb", name="eb_sb")
            for j in range(16):
                nc.sync.dma_start(
                    out=eb_sb[:, N * j:N * (j + 1)],
                    in_=ebt[128 * j:128 * (j + 1), :],
                )

            qb = singles.tile([128, N], bf16, tag="qb", name="qb")
            kb = singles.tile([128, N], bf16, tag="kb", name="kb")
            vt = singles.tile([128, N], bf16, tag="vt", name="vt")
            vsb = [singles.tile([128, 16 * VSTRIDE], bf16, tag=f"vsb{b}", name=f"vsb{b}")
                   for b in range(B)]
            for b in range(B):
                nc.vector.memset(vsb[b], 1.0)

            # ---- projection phase ----
            with tc.tile_pool(name="psP", bufs=6, space="PSUM") as psP:
                for mt in range(3):  # 0: q|qrot, 1: k|krot, 2: v|pad
                    for half in range(2):
                        chunks = [4 * half + c for c in range(4)]
                        tiles = [psP.tile([128, 512], f32, tag="s",
                                          name=f"pp_{mt}_{half}_{ci}")
                                 for ci in range(4)]
                        for k in range(4):
                            for ci, c in enumerate(chunks):
                                nc.tensor.matmul(
                                    tiles[ci],
                                    wl[k][:, 128 * mt:128 * (mt + 1)],
                                    xb[k][:, 512 * c:512 * (c + 1)],
                                    start=(k == 0), stop=(k == 3),
                                )
                        for ci, c in enumerate(chunks):
                            b = c // 4
                            tok = 512 * (c % 4)
                            if mt < 2:
                                t1 = t12p.tile([64, 512], f32, tag="t1",
                                               name=f"t1_{mt}_{c}")
                                t2 = t12p.tile([64, 512], f32, tag="t2",
                                               name=f"t2_{mt}_{c}")
                                nc.vector.tensor_mul(t1, tiles[ci][0:64, :],
                                                     cs_sb[0:64, tok:tok + 512])
                                nc.vector.tensor_mul(t2, tiles[ci][64:128, :],
                                                     cs_sb[64:128, tok:tok + 512])
                                dst = qb if mt == 0 else kb
                                nc.gpsimd.tensor_add(
                                    dst[64 * b:64 * b + 64, tok:tok + 512],
                                    t1, t2)
                            else:
                                nc.vector.tensor_copy(
                                    vt[64 * b:64 * b + 64, tok:tok + 512],
                                    tiles[ci][0:64, :])

            # V natural layout via DMA xbar transposes (per 128-token block;
            # 32B-aligned destinations): vt[64b:64b+64, jblk] -> vsb[b] block
            for b in range(B):
                for j in range(16):
                    nc.sync.dma_start_transpose(
                        vsb[b][:, VSTRIDE * j:VSTRIDE * j + 64],
                        vt[64 * b:64 * b + 64, 128 * j:128 * (j + 1)],
                    )

            if _DEBUG:
                nc.sync.dma_start(out=dbg_qkv[0, :, :], in_=qb)
                nc.sync.dma_start(out=dbg_qkv[1, :, :], in_=kb)
                nc.sync.dma_start(out=dbg_qkv[2, :, :], in_=vt)
                for b in range(B):
                    nc.sync.dma_start(out=dbg_vsb[b, :, :], in_=vsb[b])

            # ---- attention ----
            # Both batches processed together per (i-quarter, j): the two
            # K=64 S matmuls land on PE row-groups 0/64 and run concurrently.
            with (
                tc.tile_pool(name="psS", bufs=2, space="PSUM") as psS,
                tc.tile_pool(name="psO", bufs=1, space="PSUM") as psO,
                tc.tile_pool(name="psY", bufs=2, space="PSUM") as psY,
            ):
                def attn_quarter(q, fillers):
                    """Emit one 512-token i-quarter (both batches); returns
                    deferred normalization + output-projection closures."""
                    i0 = 512 * q
                    fill_iter = iter(fillers)

                    def emit_fill():
                        f = next(fill_iter, None)
                        if f is not None:
                            f()

                    LAG = 3
                    ots = [psO.tile([65, 512], f32, tag=f"o{b}", name=f"ot_{b}_{q}")
                           for b in range(B)]
                    pt_tiles = {}
                    for step in range(16 + LAG):
                        if step < 16:
                            j = step
                            s_ps = psS.tile([128, 1024], f32, tag="s",
                                            name=f"s_{q}_{j}")
                            for b in range(B):
                                nc.tensor.matmul(
                                    s_ps[:, 512 * b:512 * (b + 1)],
                                    kb[64 * b:64 * b + 64, 128 * j:128 * (j + 1)],
                                    qb[64 * b:64 * b + 64, i0:i0 + 512],
                                    start=True, stop=True,
                                )
                            pts = ptsp.tile([128, 1024], bf16, tag="pts",
                                            name=f"pts_{q}_{j}")
                            nc.scalar.activation(pts, s_ps, Exp)
                            pt = ptp.tile([128, 1024], bf16, tag="pt",
                                          name=f"pt_{q}_{j}")
                            # one 2x-rate mult: the eb block is shared by the
                            # two batch halves via a stride-0 middle dim
                            ebs = eb_sb[:, N * j + i0:N * j + i0 + 512]
                            eng = nc.gpsimd if j % 3 == 1 else nc.vector
                            eng.tensor_mul(
                                pt.rearrange("p (r c) -> p r c", r=2),
                                pts.rearrange("p (r c) -> p r c", r=2),
                                ebs.unsqueeze(1).broadcast_to((128, 2, 512)))
                            pt_tiles[j] = pt
                            if _DEBUG and q == 0 and j == 0:
                                nc.sync.dma_start(out=dbg_pt[:, :], in_=pt)
                        emit_fill()
                        if step >= LAG:
                            j = step - LAG
                            for b in range(B):
                                nc.tensor.matmul(
                                    ots[b],
                                    vsb[b][:, VSTRIDE * j:VSTRIDE * j + 65],
                                    pt_tiles[j][:, 512 * b:512 * (b + 1)],
                                    start=(j == 0), stop=(j == 15),
                                )
                            pt_tiles[j] = None
                    for f in fill_iter:
                        f()

                    # rowsums ship to the host (f32); y goes out unnormalized
                    # in bf16 and the host divides by the per-head rowsum.
                    deferred = []
                    for b in range(B):
                        ot = ots[b]
                        rs = rrp.tile([1, 512], f32, tag="rs", name=f"rs_{b}_{q}")
                        nc.vector.tensor_copy(rs, ot[64:65, :])
                        nc.sync.dma_start(out=rsum[4 * b + q:4 * b + q + 1, :],
                                          in_=rs)
                        otsb = otp.tile([64, 512], bf16, tag=f"otsb{b}",
                                        name=f"otsb_{b}_{q}")
                        nc.vector.tensor_copy(otsb, ot[0:64, :])

                        def mk_y(blk, otsb=otsb, b=b):
                            def f():
                                y_ps = psY.tile([128, 512], f32, tag="y",
                                                name=f"y_{b}_{q}_{blk}")
                                nc.tensor.matmul(
                                    y_ps, otsb[:, 128 * blk:128 * (blk + 1)],
                                    wo_sb, start=True, stop=True)
                                y_sb = ysp.tile([128, 512], bf16, tag="ysb",
                                                name=f"ysb_{b}_{q}_{blk}")
                                if blk % 4 == 3:
                                    nc.scalar.activation(y_sb, y_ps, Copy)
                                else:
                                    nc.vector.tensor_copy(y_sb, y_ps)
                                nc.sync.dma_start(
                                    out=out[b, i0 + 128 * blk:i0 + 128 * (blk + 1), :],
                                    in_=y_sb)
                            return f

                        deferred += [mk_y(blk) for blk in range(4)]
                    return deferred

                deferred = []
                for q in range(4):
                    deferred = attn_quarter(q, deferred)
                for f in deferred:
                    f()

    nc.compile()
    return nc


def _host_inputs(x, pos_bias, w_qkv, w_out):
    """Build the per-core input maps (head-parallel sharding)."""
    bf = ml_dtypes.bfloat16
    x = np.asarray(x, dtype=np.float32)
    pos_bias = np.asarray(pos_bias, dtype=np.float32)
    w_qkv = np.asarray(w_qkv, dtype=np.float32)
    w_out = np.asarray(w_out, dtype=np.float32)
    hidden = HEADS * DIM_HEAD

    xt = np.ascontiguousarray(
        np.concatenate([x[0].T, x[1].T], axis=1)).astype(bf)  # [512, 4096]

    inv_freq = 1.0 / (ROPE_THETA ** (np.arange(0, DIM_HEAD, 2, dtype=np.float64) / DIM_HEAD))
    freqs = np.arange(N, dtype=np.float64)[:, None] * inv_freq[None, :]
    freqs = np.repeat(freqs, 2, axis=-1)  # [n, 64]
    cosT = np.cos(freqs).T.astype(np.float32)
    sinT = np.sin(freqs).T.astype(np.float32)
    cs2 = np.ascontiguousarray(np.concatenate([cosT, sinT], axis=0))  # [128, n]

    def rot_cols(w):
        wr = np.empty_like(w)
        wr[:, 0::2] = -w[:, 1::2]
        wr[:, 1::2] = w[:, 0::2]
        return wr

    scale = DIM_HEAD ** -0.5
    in_maps = []
    for h in range(HEADS):
        wq = w_qkv[:, h * 64:(h + 1) * 64] * scale
        wk = w_qkv[:, hidden + h * 64:hidden + (h + 1) * 64]
        wvh = w_qkv[:, 2 * hidden + h * 64:2 * hidden + (h + 1) * 64]
        wall = np.ascontiguousarray(
            np.concatenate(
                [wq, rot_cols(wq), wk, rot_cols(wk), wvh,
                 np.zeros((DIM, 64), dtype=np.float32)], axis=1)
        ).astype(bf)  # [512, 384]
        in_maps.append({
            "xt": xt,
            "wall": wall,
            "cs2": cs2,
            "ebt": np.ascontiguousarray(np.exp(pos_bias[h]).T).astype(bf),
            "wo": np.ascontiguousarray(w_out[h * 64:(h + 1) * 64, :]).astype(bf),
        })
    return in_maps


def kernel(x, pos_bias, w_qkv, w_out, _want_trace=False):
    global _compiled
    from concourse.bass_utils import run_bass_kernel_spmd

    if _compiled is None:
        _compiled = _build()
    in_maps = _host_inputs(x, pos_bias, w_qkv, w_out)
    res = run_bass_kernel_spmd(
        _compiled, in_maps, core_ids=list(range(HEADS)), trace=_want_trace
    )
    y = np.zeros((B, N, DIM), dtype=np.float32)
    for r in res.results:
        rs = np.asarray(r["rsum"]).reshape(B, N)
        y += r["out"].astype(np.float32) / rs[:, :, None]
    if _want_trace:
        kernel._last_results = res
    return y


# revision 30
# speedup vs baseline: 1.1819x; 1.1819x over previous
"""Trainium2 Bass kernel for nn_Attention_81655918231876.

RoPE attention with positional bias, 8 heads / dim_head 64, b=2, n=2048, dim=512.
Sharding: head-parallel across 8 cores. Core h computes head h for BOTH batches
and emits a partial output y_h = softmax(q_h k_h^T + bias_h) v_h @ w_out[h-slice].
The host sums the 8 partials.

v2 design (all-bf16 matmul path; ~50x accuracy headroom under the 2e-2 gate):
  - Projections: stationary weight blocks [q|qrot], [k|krot], [v|pad] in bf16,
    moving x^T chunks; RoPE combine = one DVE mul with a stacked cos/sin table
    plus one DVE add that writes bf16 q/k packs ([b0;b1] on partitions).
  - S = q k^T as plain bf16 K=64 matmuls (tile_position rows 0/64 pick the
    batch half of the packed q/k tiles).
  - exp(S) on ScalarE -> bf16; bias multiply exp(S)*exp(bias) on DVE at 2x
    bf16 rate against a resident bf16 exp(bias^T) table loaded once.
  - P V accumulated with an extra ones-column in V so row 64 of O^T is the
    softmax row sum; 1/sum broadcast across partitions (gpsimd) and folded
    into O^T before the output projection; y DMA'd straight from PSUM.
  - V natural layout built with DMA xbar transposes (no PE transposes).
  - S -> exp -> mult -> PV software-pipelined with a 2-step lag so the PE
    stream never waits on ScalarE/DVE (keeps the PE p-state at full clock).
"""

import numpy as np
import ml_dtypes
import sys

sys.path.insert(0, "/opt/trn_rl_repo")

HEADS = 8
DIM_HEAD = 64
ROPE_THETA = 10000.0
B, N, DIM = 2, 2048, 512
# per-j-block column stride in vsb: 64 V cols + 1 ones col + pad. Must keep
# every block's byte offset 32B-aligned: the DMA xbar transpose writes in
# 16-element (bf16) groups and silently corrupts unaligned destinations.
VSTRIDE = 80

_compiled = None
_DEBUG = False


def _build():
    import concourse.bass as bass
    import concourse.tile as tile
    from concourse import bacc, mybir

    f32 = mybir.dt.float32
    bf16 = mybir.dt.bfloat16
    Exp = mybir.ActivationFunctionType.Exp
    Copy = mybir.ActivationFunctionType.Copy

    nc = bacc.Bacc(None, target_bir_lowering=False, debug=False)
    xt = nc.dram_tensor("xt", [DIM, 2 * N], bf16, kind="ExternalInput")
    wall = nc.dram_tensor("wall", [DIM, 384], bf16, kind="ExternalInput")
    cs2 = nc.dram_tensor("cs2", [128, N], f32, kind="ExternalInput")
    ebt = nc.dram_tensor("ebt", [N, N], bf16, kind="ExternalInput")
    wo = nc.dram_tensor("wo", [64, DIM], bf16, kind="ExternalInput")
    out = nc.dram_tensor("out", [B, N, DIM], bf16, kind="ExternalOutput")
    rsum = nc.dram_tensor("rsum", [4 * B, 512], f32, kind="ExternalOutput")
    if _DEBUG:
        dbg_qkv = nc.dram_tensor("dbg_qkv", [3, 128, N], bf16, kind="ExternalOutput")
        dbg_vsb = nc.dram_tensor("dbg_vsb", [B, 128, 16 * VSTRIDE], bf16, kind="ExternalOutput")
        dbg_pt = nc.dram_tensor("dbg_pt", [128, 1024], bf16, kind="ExternalOutput")
        dbg_rb = nc.dram_tensor("dbg_rb", [64, 512], f32, kind="ExternalOutput")

    with tile.TileContext(nc) as tc:
        with (
            tc.tile_pool(name="singles", bufs=1) as singles,
            tc.tile_pool(name="t12p", bufs=3) as t12p,
            tc.tile_pool(name="ptsp", bufs=3) as ptsp,
            tc.tile_pool(name="ptp", bufs=6) as ptp,
            tc.tile_pool(name="rrp", bufs=2) as rrp,
            tc.tile_pool(name="otp", bufs=2) as otp,
            tc.tile_pool(name="ysp", bufs=3) as ysp,
        ):
            # ---- constants / inputs ----
            wl = [singles.tile([128, 384], bf16, tag=f"wl{k}", name=f"wl{k}") for k in range(4)]
            for k in range(4):
                nc.sync.dma_start(out=wl[k], in_=wall[128 * k:128 * (k + 1), :])
            xb = [singles.tile([128, 2 * N], bf16, tag=f"xb{k}", name=f"xb{k}") for k in range(4)]
            for half in range(2):
                for k in range(4):
                    nc.sync.dma_start(
                        out=xb[k][:, N * half:N * (half + 1)],
                        in_=xt[128 * k:128 * (k + 1), N * half:N * (half + 1)],
                    )
            cs_sb = singles.tile([128, N], f32, tag="cs", name="cs_sb")
            nc.sync.dma_start(out=cs_sb, in_=cs2[:, :])
            wo_sb = singles.tile([64, DIM], bf16, tag="wo", name="wo_sb")
            nc.sync.dma_start(out=wo_sb, in_=wo[:, :])
            eb_sb = singles.tile([128, 16 * N], bf16, tag="eb", name="eb_sb")
            for j in range(16):
                nc.sync.dma_start(
                    out=eb_sb[:, N * j:N * (j + 1)],
                    in_=ebt[128 * j:128 * (j + 1), :],
                )

            qb = singles.tile([128, N], bf16, tag="qb", name="qb")
            kb = singles.tile([128, N], bf16, tag="kb", name="kb")
            vt = singles.tile([128, N], bf16, tag="vt", name="vt")
            vsb = [singles.tile([128, 16 * VSTRIDE], bf16, tag=f"vsb{b}", name=f"vsb{b}")
                   for b in range(B)]
            for b in range(B):
                nc.vector.memset(vsb[b], 1.0)

            # ---- projection phase ----
            with tc.tile_pool(name="psP", bufs=6, space="PSUM") as psP:
                for mt in range(3):  # 0: q|qrot, 1: k|krot, 2: v|pad
                    for half in range(2):
                        chunks = [4 * half + c for c in range(4)]
                        tiles = [psP.tile([128, 512], f32, tag="s",
                                          name=f"pp_{mt}_{half}_{ci}")
                                 for ci in range(4)]
                        for k in range(4):
                            for ci, c in enumerate(chunks):
                                nc.tensor.matmul(
                                    tiles[ci],
                                    wl[k][:, 128 * mt:128 * (mt + 1)],
                                    xb[k][:, 512 * c:512 * (c + 1)],
                                    start=(k == 0), stop=(k == 3),
                                )
                        for ci, c in enumerate(chunks):
                            b = c // 4
                            tok = 512 * (c % 4)
                            if mt < 2:
                                t1 = t12p.tile([64, 512], f32, tag="t1",
                                               name=f"t1_{mt}_{c}")
                                t2 = t12p.tile([64, 512], f32, tag="t2",
                                               name=f"t2_{mt}_{c}")
                                nc.vector.tensor_mul(t1, tiles[ci][0:64, :],
                                                     cs_sb[0:64, tok:tok + 512])
                                nc.vector.tensor_mul(t2, tiles[ci][64:128, :],
                                                     cs_sb[64:128, tok:tok + 512])
                                dst = qb if mt == 0 else kb
                                nc.gpsimd.tensor_add(
                                    dst[64 * b:64 * b + 64, tok:tok + 512],
                                    t1, t2)
                            else:
                                nc.vector.tensor_copy(
                                    vt[64 * b:64 * b + 64, tok:tok + 512],
                                    tiles[ci][0:64, :])

            # V natural layout via one DMA xbar transpose per batch:
            # vt[64b:64b+64, :] -> vsb[b] 3D view [128, 16 j, 64]
            # (ones column at VSTRIDE*j + 64 from the memset; block offsets
            # stay 32B-aligned, which the xbar requires)
            for b in range(B):
                dst = vsb[b].rearrange("p (j c) -> p j c", c=VSTRIDE)[:, :, 0:64]
                nc.sync.dma_start_transpose(dst, vt[64 * b:64 * b + 64, :])

            if _DEBUG:
                nc.sync.dma_start(out=dbg_qkv[0, :, :], in_=qb)
                nc.sync.dma_start(out=dbg_qkv[1, :, :], in_=kb)
                nc.sync.dma_start(out=dbg_qkv[2, :, :], in_=vt)
                for b in range(B):
                    nc.sync.dma_start(out=dbg_vsb[b, :, :], in_=vsb[b])

            # ---- attention ----
            # Both batches processed together per (i-quarter, j): the two
            # K=64 S matmuls land on PE row-groups 0/64 and run concurrently.
            with (
                tc.tile_pool(name="psS", bufs=2, space="PSUM") as psS,
                tc.tile_pool(name="psO", bufs=1, space="PSUM") as psO,
                tc.tile_pool(name="psY", bufs=2, space="PSUM") as psY,
            ):
                def attn_quarter(q, fillers):
                    """Emit one 512-token i-quarter (both batches); returns
                    deferred normalization + output-projection closures."""
                    i0 = 512 * q
                    fill_iter = iter(fillers)

                    def emit_fill():
                        f = next(fill_iter, None)
                        if f is not None:
                            f()

                    LAG = 3
                    ots = [psO.tile([65, 512], f32, tag=f"o{b}", name=f"ot_{b}_{q}")
                           for b in range(B)]
                    pt_tiles = {}
                    for step in range(16 + LAG):
                        if step < 16:
                            j = step
                            s_ps = psS.tile([128, 1024], f32, tag="s",
                                            name=f"s_{q}_{j}")
                            for b in range(B):
                                nc.tensor.matmul(
                                    s_ps[:, 512 * b:512 * (b + 1)],
                                    kb[64 * b:64 * b + 64, 128 * j:128 * (j + 1)],
                                    qb[64 * b:64 * b + 64, i0:i0 + 512],
                                    start=True, stop=True,
                                )
                            pts = ptsp.tile([128, 1024], bf16, tag="pts",
                                            name=f"pts_{q}_{j}")
                            nc.scalar.activation(pts, s_ps, Exp)
                            pt = ptp.tile([128, 1024], bf16, tag="pt",
                                          name=f"pt_{q}_{j}")
                            # one 2x-rate mult: the eb block is shared by the
                            # two batch halves via a stride-0 middle dim
                            ebs = eb_sb[:, N * j + i0:N * j + i0 + 512]
                            eng = nc.vector
                            eng.tensor_mul(
                                pt.rearrange("p (r c) -> p r c", r=2),
                                pts.rearrange("p (r c) -> p r c", r=2),
                                ebs.unsqueeze(1).broadcast_to((128, 2, 512)))
                            pt_tiles[j] = pt
                            if _DEBUG and q == 0 and j == 0:
                                nc.sync.dma_start(out=dbg_pt[:, :], in_=pt)
                        emit_fill()
                        if step >= LAG:
                            j = step - LAG
                            for b in range(B):
                                nc.tensor.matmul(
                                    ots[b],
                                    vsb[b][:, VSTRIDE * j:VSTRIDE * j + 65],
                                    pt_tiles[j][:, 512 * b:512 * (b + 1)],
                                    start=(j == 0), stop=(j == 15),
                                )
                            pt_tiles[j] = None
                    for f in fill_iter:
                        f()

                    # rowsums ship to the host (f32); y goes out unnormalized
                    # in bf16 and the host divides by the per-head rowsum.
                    deferred = []
                    for b in range(B):
                        ot = ots[b]
                        rs = rrp.tile([1, 512], f32, tag="rs", name=f"rs_{b}_{q}")
                        nc.vector.tensor_copy(rs, ot[64:65, :])
                        nc.sync.dma_start(out=rsum[4 * b + q:4 * b + q + 1, :],
                                          in_=rs)
                        otsb = otp.tile([64, 512], bf16, tag=f"otsb{b}",
                                        name=f"otsb_{b}_{q}")
                        nc.vector.tensor_copy(otsb, ot[0:64, :])

                        def mk_y(blk, otsb=otsb, b=b):
                            def f():
                                y_ps = psY.tile([128, 512], f32, tag="y",
                                                name=f"y_{b}_{q}_{blk}")
                                nc.tensor.matmul(
                                    y_ps, otsb[:, 128 * blk:128 * (blk + 1)],
                                    wo_sb, start=True, stop=True)
                                y_sb = ysp.tile([128, 512], bf16, tag="ysb",
                                                name=f"ysb_{b}_{q}_{blk}")
                                if blk % 4 == 3:
                                    nc.scalar.activation(y_sb, y_ps, Copy)
                                else:
                                    nc.vector.tensor_copy(y_sb, y_ps)
                                nc.sync.dma_start(
                                    out=out[b, i0 + 128 * blk:i0 + 128 * (blk + 1), :],
                                    in_=y_sb)
                            return f

                        deferred += [mk_y(blk) for blk in range(4)]
                    return deferred

                deferred = []
                for q in range(4):
                    deferred = attn_quarter(q, deferred)
                for f in deferred:
                    f()

    nc.compile()
    return nc


def _host_inputs(x, pos_bias, w_qkv, w_out):
    """Build the per-core input maps (head-parallel sharding)."""
    bf = ml_dtypes.bfloat16
    x = np.asarray(x, dtype=np.float32)
    pos_bias = np.asarray(pos_bias, dtype=np.float32)
    w_qkv = np.asarray(w_qkv, dtype=np.float32)
    w_out = np.asarray(w_out, dtype=np.float32)
    hidden = HEADS * DIM_HEAD

    xt = np.ascontiguousarray(
        np.concatenate([x[0].T, x[1].T], axis=1)).astype(bf)  # [512, 4096]

    inv_freq = 1.0 / (ROPE_THETA ** (np.arange(0, DIM_HEAD, 2, dtype=np.float64) / DIM_HEAD))
    freqs = np.arange(N, dtype=np.float64)[:, None] * inv_freq[None, :]
    freqs = np.repeat(freqs, 2, axis=-1)  # [n, 64]
    cosT = np.cos(freqs).T.astype(np.float32)
    sinT = np.sin(freqs).T.astype(np.float32)
    cs2 = np.ascontiguousarray(np.concatenate([cosT, sinT], axis=0))  # [128, n]

    def rot_cols(w):
        wr = np.empty_like(w)
        wr[:, 0::2] = -w[:, 1::2]
        wr[:, 1::2] = w[:, 0::2]
        return wr

    scale = DIM_HEAD ** -0.5
    in_maps = []
    for h in range(HEADS):
        wq = w_qkv[:, h * 64:(h + 1) * 64] * scale
        wk = w_qkv[:, hidden + h * 64:hidden + (h + 1) * 64]
        wvh = w_qkv[:, 2 * hidden + h * 64:2 * hidden + (h + 1) * 64]
        wall = np.ascontiguousarray(
            np.concatenate(
                [wq, rot_cols(wq), wk, rot_cols(wk), wvh,
                 np.zeros((DIM, 64), dtype=np.float32)], axis=1)
        ).astype(bf)  # [512, 384]
        in_maps.append({
            "xt": xt,
            "wall": wall,
            "cs2": cs2,
            "ebt": np.ascontiguousarray(np.exp(pos_bias[h]).T).astype(bf),
            "wo": np.ascontiguousarray(w_out[h * 64:(h + 1) * 64, :]).astype(bf),
        })
    return in_maps


def kernel(x, pos_bias, w_qkv, w_out, _want_trace=False):
    global _compiled
    from concourse.bass_utils import run_bass_kernel_spmd

    if _compiled is None:
        _compiled = _build()
    in_maps = _host_inputs(x, pos_bias, w_qkv, w_out)
    res = run_bass_kernel_spmd(
        _compiled, in_maps, core_ids=list(range(HEADS)), trace=_want_trace
    )
    y = np.zeros((B, N, DIM), dtype=np.float32)
    for r in res.results:
        rs = np.asarray(r["rsum"]).reshape(B, N)
        y += r["out"].astype(np.float32) / rs[:, :, None]
    if _want_trace:
        kernel._last_results = res
    return y
